# revision 28
# baseline (speedup 1.0000x reference)
"""Trainium2 Bass kernel for Llama-style GQA causal self-attention.

Problem (hardcoded): T=4096, HID=2048, D=128, NQ=16, NKV=4, rotate-half RoPE,
causal softmax, o_proj.  8 NeuronCores, tensor-parallel over heads:

  core c: Q heads {2c, 2c+1}, KV head c//2.

Single fused pipeline: per 512-row block tb the kernel emits
  projections (q h0/h1, k, v) -> RoPE -> attention(h1, tb-3) -> attention(h0, tb)
so the tensor engine never sees a phase boundary and ScalarE exp overlaps
projection matmuls.  Head-1 attention lags head-0 by 3 blocks; its last three
blocks run after the head-0 AllToAll fires, hiding the collective, and the
head-1 AllToAll hides under o_proj head-0.

Attention inner loop is software-pipelined: the score matmul for chunk j+2
issues before the PV matmuls of chunk j, so PV never waits on ScalarE exp.
PV uses pt (exp scores) as the *stationary* operand and the augmented
v rows [v_d0..63 | 1 | v_d64..127] as the 129-wide moving operand, producing
po in [q, d] orientation with the softmax denominator landing in column 64.
Normalization is then per-partition tensor_scalar multiplies (no partition
broadcast / wide reciprocal).  The AllToAll carries [T-rows, D] tiles; the
o_proj input is transposed back to [D, rows] by DMA-transpose during load.

RoPE uses a sign-folded sin table ([-sin; +sin]): 2 ScalarE half-copies (the
partition-crossing moves) + 3 full-width DVE ops.  Causal masking is a DVE
multiply with a static lower-triangle tile.  wo streams through a 4-buffer
pool so both o_proj passes reuse the same resident tiles.  Head-1 o_proj
writes a second output tensor summed with head-0's on the host.

Host-side prep is layout only: transpose + bf16-cast of weights/activations,
RoPE cos/sin tables from position_ids, sharding, final concat + add.
"""

import numpy as np
import ml_dtypes

import concourse.bass as bass
import concourse.mybir as mybir
import concourse.tile as tile
from concourse import bacc
from concourse.bass_utils import run_bass_kernel_spmd
from concourse.masks import make_identity

T, HID, D = 4096, 2048, 128
NQ, NKV = 16, 4
THETA = 10000.0
NCORES = 8
HPC = NQ // NCORES        # q heads per core = 2
TB = 512                  # t block
NT = T // TB              # 8
KC = HID // 128           # 16 contraction chunks
HALF = D // 2             # 64
LAG = 2                   # head-1 attention lags head-0 by this many blocks
NQC = TB // 128           # q sub-chunks per block = 4
SCALE = 1.0 / float(np.sqrt(D))
FP32 = mybir.dt.float32
BF16 = mybir.dt.bfloat16
NPBF16 = ml_dtypes.bfloat16


def _build_nc():
    nc = bacc.Bacc("TRN2", num_devices=NCORES)

    hsT = nc.declare_dram_parameter("hsT", [HID, T], BF16, isOutput=False)
    wqT = nc.declare_dram_parameter("wqT", [HID, HPC * D], BF16, isOutput=False)
    wkT = nc.declare_dram_parameter("wkT", [HID, D], BF16, isOutput=False)
    wvT = nc.declare_dram_parameter("wvT", [HID, D], BF16, isOutput=False)
    woT = nc.declare_dram_parameter("woT", [HID, HID], BF16, isOutput=False)
    cosT = nc.declare_dram_parameter("cosT", [D, T], BF16, isOutput=False)
    sinT = nc.declare_dram_parameter("sinT", [D, T], BF16, isOutput=False)
    outp = nc.declare_dram_parameter("out", [TB, HID], FP32, isOutput=True)
    outp2 = nc.declare_dram_parameter("out2", [TB, HID], FP32, isOutput=True)

    # per-head collective bounce buffers (internal DRAM), [rows, d] layout
    a2a_in = [nc.dram_tensor(f"a2a_in{h}", [NCORES, TB, D], BF16) for h in range(HPC)]
    a2a_out = [nc.dram_tensor(f"a2a_out{h}", [NCORES, TB, D], BF16) for h in range(HPC)]

    hsT_r = hsT.rearrange("(c p) t -> p c t", p=128)
    wqT_r = wqT.rearrange("(c p) m -> p c m", p=128)
    wkT_r = wkT.rearrange("(c p) m -> p c m", p=128)
    wvT_r = wvT.rearrange("(c p) m -> p c m", p=128)
    woT_r = woT.rearrange("(c p) m -> p c m", p=128)
    # destination-block view for the attention-output writes: [q, slot, qc, d]
    a2a_in_r = [x.rearrange("r (qc q) d -> q r qc d", qc=NQC) for x in a2a_in]

    with tile.TileContext(nc) as tc:
        with (
            tc.tile_pool(name="const", bufs=1) as cpool,
            tc.tile_pool(name="hsx", bufs=2) as hpool,
            tc.tile_pool(name="qblk", bufs=4) as qpool,
            tc.tile_pool(name="kv", bufs=1) as kvpool,
            tc.tile_pool(name="pt", bufs=8) as ptpool,
            tc.tile_pool(name="rp", bufs=2) as rppool,
            tc.tile_pool(name="tmp", bufs=4) as tpool,
            tc.tile_pool(name="att", bufs=3) as apool,
            tc.tile_pool(name="rec", bufs=2) as rpool,
            tc.tile_pool(name="wos", bufs=4) as wpool,
            tc.tile_pool(name="afp", bufs=2) as afpool,
            tc.tile_pool(name="osb", bufs=4) as opool,
            tc.tile_pool(name="ps", bufs=3, space="PSUM") as ps,
            tc.tile_pool(name="acc", bufs=4, space="PSUM") as psacc,
            tc.tile_pool(name="lsum", bufs=1, space="PSUM") as pslsum,
        ):
            # ---- constants / weights (wo streamed later in phase 4) ----
            wq_sb = cpool.tile([128, KC, HPC * D], BF16, tag="wq")
            wk_sb = cpool.tile([128, KC, D], BF16, tag="wk")
            wv_sb = cpool.tile([128, KC, D], BF16, tag="wv")
            cos_sb = cpool.tile([D, T], BF16, tag="cos")
            sin_sb = cpool.tile([D, T], BF16, tag="sin")
            mask_sb = cpool.tile([128, TB], BF16, tag="mask")
            ident = cpool.tile([128, 128], BF16, tag="ident")
            kT = kvpool.tile([128, T], BF16, tag="kT")
            # v rows augmented with a ones column: [v_d0..63 | 1 | v_d64..127]
            # (129-wide moving operand for PV; column 64 accumulates the
            # softmax denominator)
            vv = kvpool.tile([128, T // 128, D + 2], BF16, tag="vv")

            for c8 in range(8):  # split across DMA queues
                cs = slice(c8 * 2, (c8 + 1) * 2)
                nc.sync.dma_start(out=wq_sb[:, cs, :], in_=wqT_r[:, cs, :])
            nc.gpsimd.memset(vv[:, :, HALF:HALF + 1], 1.0)
            # static causal mask: 1 where q-offset (free) >= k-offset (partition)
            mtmp = tpool.tile([128, TB], BF16, tag="tmp")
            nc.gpsimd.memset(mtmp[:, :], 1.0)
            nc.gpsimd.affine_select(
                out=mask_sb[:, :], in_=mtmp[:, :],
                compare_op=mybir.AluOpType.is_ge,
                fill=0.0, base=0,
                pattern=[[1, TB]], channel_multiplier=-1,
            )
            make_identity(nc, ident[:, :])

            qblks = {}

            def rope(dst, srcp, ts):
                # dst = src*cos + [x2; x1]*[-sin; +sin]  (sign folded into table)
                rot2 = rppool.tile([128, TB], BF16, tag="rot")
                nc.scalar.copy(rot2[0:HALF, :], srcp[HALF:128, :])
                nc.scalar.copy(rot2[HALF:128, :], srcp[0:HALF, :])
                t1 = rppool.tile([128, TB], BF16, tag="rt1")
                t2 = rppool.tile([128, TB], BF16, tag="rt2")
                nc.vector.tensor_tensor(t1, srcp, cos_sb[:, ts], mybir.AluOpType.mult)
                nc.vector.tensor_tensor(t2, rot2, sin_sb[:, ts], mybir.AluOpType.mult)
                nc.vector.tensor_tensor(dst, t1, t2, mybir.AluOpType.add)

            def proj_step(tb):
                ts = slice(tb * TB, (tb + 1) * TB)
                hsx = hpool.tile([128, KC, TB], BF16, tag="hsx")
                for c8 in range(8):  # split across DMA queues
                    cs = slice(c8 * 2, (c8 + 1) * 2)
                    nc.sync.dma_start(out=hsx[:, cs, :], in_=hsT_r[:, cs, ts])
                if tb == 0:
                    # secondary weights, after the critical wq+hsx0 loads
                    nc.sync.dma_start(out=cos_sb[:, 0:T // 2], in_=cosT[:, 0:T // 2])
                    nc.sync.dma_start(out=sin_sb[:, 0:T // 2], in_=sinT[:, 0:T // 2])
                    nc.sync.dma_start(out=cos_sb[:, T // 2:T], in_=cosT[:, T // 2:T])
                    nc.sync.dma_start(out=sin_sb[:, T // 2:T], in_=sinT[:, T // 2:T])
                    nc.sync.dma_start(out=wk_sb[:, :, :], in_=wkT_r)
                    nc.sync.dma_start(out=wv_sb[:, :, :], in_=wvT_r)

                qblk = qpool.tile([128, HPC, TB], BF16, tag="qblk")
                qblks[tb] = qblk
                for h in range(HPC):
                    qps = ps.tile([128, TB], FP32, tag="mm512")
                    for c in range(KC):
                        nc.tensor.matmul(
                            qps[:, :],
                            lhsT=wq_sb[:, c, h * D:(h + 1) * D],
                            rhs=hsx[:, c, :],
                            start=(c == 0), stop=(c == KC - 1),
                        )
                    rope(qblk[:, h, :], qps, ts)

                kps = ps.tile([128, TB], FP32, tag="mm512")
                for c in range(KC):
                    nc.tensor.matmul(
                        kps[:, :], lhsT=wk_sb[:, c, :], rhs=hsx[:, c, :],
                        start=(c == 0), stop=(c == KC - 1),
                    )
                rope(kT[:, ts], kps, ts)

                # v computed transposed ([d, t]), then flipped to [t, d] on the PE
                vps = ps.tile([128, TB], FP32, tag="mm512")
                for c in range(KC):
                    nc.tensor.matmul(
                        vps[:, :], lhsT=wv_sb[:, c, :], rhs=hsx[:, c, :],
                        start=(c == 0), stop=(c == KC - 1),
                    )
                vtw = tpool.tile([128, TB], BF16, tag="tmp")
                nc.vector.tensor_copy(vtw[:, :], vps[:, :])
                vtp = pslsum.tile([128, TB // 128, 128], BF16, tag="lsum")
                for tt in range(TB // 128):
                    nc.tensor.transpose(
                        vtp[:, tt, :],
                        vtw[:, tt * 128:(tt + 1) * 128],
                        ident[:, :],
                    )
                jb = slice(tb * 4, (tb + 1) * 4)
                nc.vector.tensor_copy(vv[:, jb, 0:HALF], vtp[:, :, 0:HALF])
                nc.vector.tensor_copy(
                    vv[:, jb, HALF + 1:D + 1], vtp[:, :, HALF:128]
                )

            def attn_block(h, i4):
                nj = 4 * i4 + 4
                # po in [q, qc, (v_lo | l | v_hi)] orientation, split in qc
                # pairs so each accumulation region stays inside one PSUM bank
                poE = psacc.tile([128, 2, D + 1], FP32, tag="acc")
                poF = psacc.tile([128, 2, D + 1], FP32, tag="acc")
                # two 516B accumulation regions share each bank, so the
                # matmul start-flag (which zero-flags a whole 2KB bank) cannot
                # be used: pre-zero on DVE and accumulate with start=False
                nc.vector.memset(poE[:, :, :], 0.0)
                nc.vector.memset(poF[:, :, :], 0.0)

                def po_region(qc):
                    return poE[:, qc, :] if qc < 2 else poF[:, qc - 2, :]

                # diagonal (masked) chunks first: their mask-mult latency
                # hides in pipeline fill instead of block tail
                jorder = list(range(4 * i4, nj)) + list(range(4 * i4))
                pts = {}
                geom = {}

                def emit_score(idx):
                    j = jorder[idx]
                    m = j - 4 * i4  # >=0 on diagonal 512-block
                    off = 128 * m if m > 0 else 0
                    w = TB - off
                    geom[idx] = (j, off, w)
                    sps = ps.tile([128, TB], FP32, tag="mm512")
                    nc.tensor.matmul(
                        sps[:, 0:w],
                        lhsT=kT[:, j * 128:(j + 1) * 128],
                        rhs=qblks[i4][:, h, off:TB],
                        start=True, stop=True,
                    )
                    pt = ptpool.tile([128, TB], BF16, tag="pt")
                    nc.scalar.activation(
                        pt[:, 0:w], sps[:, 0:w],
                        mybir.ActivationFunctionType.Exp, scale=SCALE,
                    )
                    if m >= 0:
                        # zero entries where q < k (within-block causality)
                        nc.vector.tensor_tensor(
                            pt[:, 0:w], pt[:, 0:w], mask_sb[:, 0:w],
                            mybir.AluOpType.mult,
                        )
                    pts[idx] = pt

                # 2-chunk score lookahead so PV never waits on ScalarE exp
                for idx in range(min(2, nj)):
                    emit_score(idx)
                for idx in range(nj):
                    if idx + 2 < nj:
                        emit_score(idx + 2)
                    j, off, w = geom[idx]
                    pt = pts.pop(idx)
                    for qc in range(off // 128, NQC):
                        nc.tensor.matmul(
                            po_region(qc),
                            lhsT=pt[:, qc * 128 - off:qc * 128 - off + 128],
                            rhs=vv[:, j, 0:D + 1],
                            start=False, stop=(idx == nj - 1),
                            skip_group_check=True,
                        )
                # normalize: per-partition scalars, no broadcast needed
                ls = rpool.tile([128, NQC], FP32, tag="recl")
                rs = rpool.tile([128, NQC], FP32, tag="recr")
                scr = rpool.tile([128, NQC], FP32, tag="recs")
                nc.vector.tensor_copy(ls[:, 0:2], poE[:, :, HALF:HALF + 1])
                nc.vector.tensor_copy(ls[:, 2:4], poF[:, :, HALF:HALF + 1])
                nc.vector.reciprocal_approx_accurate(
                    out=rs[:, :], in_=ls[:, :], scratch=scr[:, :]
                )
                at = apool.tile([128, NQC, D], BF16, tag="attnT")
                for qc in range(NQC):
                    po = po_region(qc)
                    nc.vector.tensor_scalar_mul(
                        at[:, qc, 0:HALF], po[:, 0:HALF], rs[:, qc:qc + 1]
                    )
                    nc.vector.tensor_scalar_mul(
                        at[:, qc, HALF:D], po[:, HALF + 1:D + 1], rs[:, qc:qc + 1]
                    )
                nc.sync.dma_start(out=a2a_in_r[h][:, i4, :, :], in_=at[:, :, :])

            # wo oo-tiles: three loads spread over late main-loop steps
            # (DMA queues are idle there), the last in the tail
            wo_tiles = {}

            def load_wo(oo):
                wt = wpool.tile([128, KC, TB], BF16, tag="wos")
                for c4 in range(4):
                    cs = slice(c4 * 4, (c4 + 1) * 4)
                    nc.sync.dma_start(
                        out=wt[:, cs, :], in_=woT_r[:, cs, oo * TB:(oo + 1) * TB]
                    )
                wo_tiles[oo] = wt

            # ---- fused pipeline ----
            for tb in range(NT):
                proj_step(tb)
                if tb >= NT - 3:
                    load_wo(tb - (NT - 3))
                if tb >= LAG:
                    attn_block(1, tb - LAG)
                attn_block(0, tb)

            nc.gpsimd.collective_compute(
                "AllToAll",
                mybir.AluOpType.bypass,
                replica_groups=[list(range(NCORES))],
                ins=[a2a_in[0][:, :, :]],
                outs=[a2a_out[0][:, :, :]],
            )

            load_wo(3)  # last wo tile during the h1 tail blocks

            attn_block(1, NT - LAG)
            # af0 gather (DMA-transpose back to [d, rows]): emitted after the
            # first tail block so its collective-wait doesn't head-of-line
            # block that block's a2a-input DMAs on the in-order sync queue,
            # but early enough to finish during the remaining tail blocks
            af0 = afpool.tile([128, NCORES, TB], BF16, tag="af")
            for r in range(NCORES):
                nc.sync.dma_start_transpose(
                    out=af0[:, r, :], in_=a2a_out[0][r, :, :]
                )
            for i4 in range(NT - LAG + 1, NT):
                attn_block(1, i4)

            nc.gpsimd.collective_compute(
                "AllToAll",
                mybir.AluOpType.bypass,
                replica_groups=[list(range(NCORES))],
                ins=[a2a_in[1][:, :, :]],
                outs=[a2a_out[1][:, :, :]],
            )
            af1 = afpool.tile([128, NCORES, TB], BF16, tag="af")
            for r in range(NCORES):
                nc.sync.dma_start_transpose(
                    out=af1[:, r, :], in_=a2a_out[1][r, :, :]
                )

            # ---- phase 4: o_proj rows, one half-contraction per head ----
            def phase4(h, af, dstp):
                for oo in range(4):
                    wt = wo_tiles[oo]
                    for tt in range(TB // 128):
                        ops_ = ps.tile([128, TB], FP32, tag="mm512")
                        for r in range(NCORES):
                            nc.tensor.matmul(
                                ops_[:, :],
                                lhsT=af[:, r, tt * 128:(tt + 1) * 128],
                                rhs=wt[:, 2 * r + h, :],
                                start=(r == 0), stop=(r == NCORES - 1),
                            )
                        osb = opool.tile([128, TB], FP32, tag="osb")
                        nc.vector.tensor_copy(osb[:, :], ops_[:, :])
                        dst = dstp[tt * 128:(tt + 1) * 128, oo * TB:(oo + 1) * TB]
                        nc.sync.dma_start(out=dst, in_=osb[:, :])

            phase4(0, af0, outp)
            phase4(1, af1, outp2)  # summed with outp on the host

    nc.finalize()
    return nc


_NC_CACHE = {}


def _get_nc():
    if "nc" not in _NC_CACHE:
        _NC_CACHE["nc"] = _build_nc()
    return _NC_CACHE["nc"]


def _prep_inputs(hidden_states, wq, wk, wv, wo, position_ids):
    hs = np.asarray(hidden_states, dtype=np.float32)
    hsT = np.ascontiguousarray(hs.T).astype(NPBF16)

    inv_freq = 1.0 / (THETA ** (np.arange(0, HALF, dtype=np.float32) / HALF))
    freqs = np.asarray(position_ids).astype(np.float32)[:, None] * inv_freq[None, :]
    cos1 = np.cos(freqs).T  # [64, T]
    sin1 = np.sin(freqs).T
    cosT = np.ascontiguousarray(np.concatenate([cos1, cos1], axis=0)).astype(NPBF16)
    # sign-folded sin: rows 0..63 multiply x2 with -sin, rows 64..127 use +sin
    sinT = np.ascontiguousarray(np.concatenate([-sin1, sin1], axis=0)).astype(NPBF16)

    woT = np.ascontiguousarray(np.asarray(wo, dtype=np.float32).T).astype(NPBF16)

    in_maps = []
    for c in range(NCORES):
        kv = c // 2
        wq_c = np.asarray(wq, dtype=np.float32)[2 * c * D:(2 * c + HPC) * D, :]
        in_maps.append({
            "hsT": hsT,
            "wqT": np.ascontiguousarray(wq_c.T).astype(NPBF16),
            "wkT": np.ascontiguousarray(
                np.asarray(wk, dtype=np.float32)[kv * D:(kv + 1) * D, :].T
            ).astype(NPBF16),
            "wvT": np.ascontiguousarray(
                np.asarray(wv, dtype=np.float32)[kv * D:(kv + 1) * D, :].T
            ).astype(NPBF16),
            "woT": woT,
            "cosT": cosT,
            "sinT": sinT,
        })
    return in_maps


def run(inputs, trace=False, tmpdir=None):
    """Run on HW; returns (output, BassKernelResults)."""
    nc = _get_nc()
    in_maps = _prep_inputs(**inputs)
    res = run_bass_kernel_spmd(
        nc, in_maps, core_ids=list(range(NCORES)), trace=trace, tmpdir=tmpdir
    )
    out = np.concatenate(
        [
            np.asarray(res.results[c]["out"], dtype=np.float32)
            + np.asarray(res.results[c]["out2"], dtype=np.float32)
            for c in range(NCORES)
        ],
        axis=0,
    )
    return out, res


def kernel(hidden_states, wq, wk, wv, wo, position_ids):
    out, _ = run(dict(
        hidden_states=hidden_states, wq=wq, wk=wk, wv=wv, wo=wo,
        position_ids=position_ids,
    ))
    return out
